# revision 1
# baseline (speedup 1.0000x reference)
"""Chamfer-distance loss kernel for Trainium2 (8 NeuronCores, SPMD).

Problem: loss = chamfer(coarse, gt_pts) + alpha * chamfer(fine, gt_pts)
  coarse [8,1024,3], fine [8,8192,3], gt [8,3,8192] (channel-first), alpha scalar.
  chamfer(x,y) = mean_n min_m d(n,m) + mean_m min_n d(n,m), d = squared L2.

Sharding: data-parallel over batch — one batch element per NeuronCore.

Per-core device pipeline (per x-family, fine and coarse):
  - d is produced 128x512 at a time by the PE as a K=9 fp16 matmul:
      lhsT rows {x0,x1,x2, 1,1,1, 1,1,1}
      rhs  rows {-2y0,-2y1,-2y2, y0^2hi,y1^2hi,y2^2hi, y0^2lo,y1^2lo,y2^2lo}
    so PSUM holds (|y|^2 - 2x.y) in fp32; |y|^2 enters at ~fp32 precision via
    the fp16 hi/lo split, and |x|^2 (a per-partition constant) is absent.
  - ScalarE casts PSUM + |x|^2-bias to an fp16 tile S (activation Identity
    with per-partition bias), so S holds fp16(d).
  - Row direction (min over m): one tensor_scalar per tile (op0=min vs a
    large constant = identity, op1=min into accum_out) — single-source, so
    it runs at fp16 4x mode. (tensor_tensor_reduce would be the natural op
    but hard-crashes the exec unit on this runtime for every dtype; GPSIMD
    tensor_tensor fails to compile — both verified by bisection.)
  - Col direction (min over n): VectorE accumulates an elementwise running
    min over S at fp16 2x mode. Partition-axis collapse at the end via PE
    transposes + free-dim reduces + ones-matmul.

Host does only O(N) prep (transpose/cast/aug-row construction) and the final
scalar arithmetic. Expected rel-err vs fp32 reference ~2e-5 to 6e-5.
"""

import sys

sys.path.insert(0, "/opt/trn_rl_repo")

import numpy as np

B = 8
NF = 8192  # fine points
NC_ = 1024  # coarse points
M = 8192  # gt points

# --- module-level program cache -------------------------------------------
_PROGRAM = None
PROFILE = False  # set True (e.g. from test.py) to capture an NTFF profile
LAST_RESULTS = None  # BassKernelResults of the most recent run


def _build_program():
    from concourse import bacc, bass, tile
    import concourse.mybir as mybir

    f16, f32 = mybir.dt.float16, mybir.dt.float32
    AL = mybir.AluOpType
    ACTF = mybir.ActivationFunctionType

    nc = bacc.Bacc("TRN2", target_bir_lowering=False, debug=False, num_devices=B)

    xaug_f = nc.dram_tensor("xaug_f", [9, NF], f16, kind="ExternalInput")
    xaug_c = nc.dram_tensor("xaug_c", [9, NC_], f16, kind="ExternalInput")
    yaug_d = nc.dram_tensor("yaug", [9, M], f16, kind="ExternalInput")
    x2f_d = nc.dram_tensor("x2f", [128, NF // 128], f32, kind="ExternalInput")
    x2c_d = nc.dram_tensor("x2c", [128, NC_ // 128], f32, kind="ExternalInput")
    iden_d = nc.dram_tensor("iden", [128, 128], f16, kind="ExternalInput")
    ones_d = nc.dram_tensor("ones128", [128, 1], f32, kind="ExternalInput")
    out_d = nc.dram_tensor("out", [1, 8], f32, kind="ExternalOutput")

    with tile.TileContext(nc) as tc:
        with (
            tc.tile_pool(name="const", bufs=1) as cpool,
            tc.tile_pool(name="s", bufs=4) as spool,
            tc.tile_pool(name="scr", bufs=2) as scrpool,
            tc.tile_pool(name="fin", bufs=1) as fpool,
            tc.tile_pool(name="ps", bufs=2, space=bass.MemorySpace.PSUM) as pspool,
        ):
            Xf = cpool.tile([9, NF], f16)
            nc.sync.dma_start(Xf[:], xaug_f.ap())
            Xc = cpool.tile([9, NC_], f16)
            nc.sync.dma_start(Xc[:], xaug_c.ap())
            Y = cpool.tile([9, M], f16)
            nc.sync.dma_start(Y[:], yaug_d.ap())
            x2f = cpool.tile([128, NF // 128], f32)
            nc.sync.dma_start(x2f[:], x2f_d.ap())
            x2c = cpool.tile([128, NC_ // 128], f32)
            nc.sync.dma_start(x2c[:], x2c_d.ap())
            iden = cpool.tile([128, 128], f16)
            nc.sync.dma_start(iden[:], iden_d.ap())
            ones = cpool.tile([128, 1], f32)
            nc.sync.dma_start(ones[:], ones_d.ap())

            outb = cpool.tile([1, 8], f32)

            accf = cpool.tile([128, M], f16)
            accc = cpool.tile([128, M], f16)
            rowWf = cpool.tile([128, NF // 128], f32)
            rowWc = cpool.tile([128, NC_ // 128], f32)

            def family(Xa, nT, acc, rowW, x2):
                for i in range(nT):
                    S = spool.tile([128, M], f16, tag="S")
                    for g in range(4):
                        ps = pspool.tile([128, 2048], f32, tag="ps")
                        for j in range(4):
                            mlo = g * 2048 + j * 512
                            nc.tensor.matmul(
                                ps[:, j * 512 : (j + 1) * 512],
                                lhsT=Xa[:, i * 128 : (i + 1) * 128],
                                rhs=Y[:, mlo : mlo + 512],
                                start=True,
                                stop=True,
                            )
                        # S = fp16(psum + |x|^2): ScalarE cast+bias, except
                        # one cast in five goes to VectorE (tensor_scalar
                        # add, PSUM src) to balance the two engines —
                        # ScalarE is otherwise the busier one.
                        if g == 0 and i % 5 == 4:
                            nc.vector.tensor_scalar(
                                out=S[:, 0:2048],
                                in0=ps[:],
                                scalar1=x2[:, i : i + 1],
                                scalar2=None,
                                op0=AL.add,
                            )
                        else:
                            nc.scalar.activation(
                                S[:, g * 2048 : (g + 1) * 2048],
                                ps[:],
                                ACTF.Identity,
                                bias=x2[:, i : i + 1],
                                scale=1.0,
                            )
                    # row-path: single-source tensor_scalar at fp16 4x mode;
                    # op0 is a no-op (min vs 60000 > any d), op1=min reduces
                    # the row into accum_out. HW-verified (bisect_hw.py s6).
                    scr = scrpool.tile([128, M], f16, tag="scr")
                    nc.vector.tensor_scalar(
                        out=scr[:],
                        in0=S[:],
                        scalar1=60000.0,
                        scalar2=None,
                        op0=AL.min,
                        op1=AL.min,
                        accum_out=rowW[:, i : i + 1],
                    )
                    if i == 0:
                        nc.vector.tensor_copy(acc[:], S[:])
                    else:
                        nc.vector.tensor_tensor(
                            out=acc[:], in0=acc[:], in1=S[:], op=AL.min
                        )

            family(Xf, NF // 128, accf, rowWf, x2f)
            family(Xc, NC_ // 128, accc, rowWc, x2c)

            def finals(acc, rowW, nT, oidx):
                # row total = sum_n min_m d(n, m)
                rsum = fpool.tile([128, 1], f32, tag=f"rsum{oidx}")
                nc.vector.tensor_reduce(
                    out=rsum[:], in_=rowW[:], axis=mybir.AxisListType.X, op=AL.add
                )
                pr = pspool.tile([1, 1], f32, tag="ps")
                nc.tensor.matmul(pr[:], lhsT=rsum[:], rhs=ones[:], start=True, stop=True)
                nc.vector.tensor_copy(outb[0:1, oidx : oidx + 1], pr[:])

                # col total = sum_m (min over partitions of acc[:, m])
                cmb = fpool.tile([128, M // 128], f32, tag=f"cmb{oidx}")
                for c0 in range(0, M // 128, 4):
                    pst = pspool.tile([128, 4, 128], f16, tag="ps")
                    for q in range(4):
                        nc.tensor.transpose(
                            pst[:, q, :],
                            acc[:, (c0 + q) * 128 : (c0 + q + 1) * 128],
                            iden[:],
                        )
                    nc.vector.tensor_reduce(
                        out=cmb[:, c0 : c0 + 4],
                        in_=pst[:],
                        axis=mybir.AxisListType.X,
                        op=AL.min,
                    )
                csum = fpool.tile([128, 1], f32, tag=f"csum{oidx}")
                nc.vector.tensor_reduce(
                    out=csum[:], in_=cmb[:], axis=mybir.AxisListType.X, op=AL.add
                )
                pc = pspool.tile([1, 1], f32, tag="ps")
                nc.tensor.matmul(pc[:], lhsT=csum[:], rhs=ones[:], start=True, stop=True)
                nc.vector.tensor_copy(outb[0:1, oidx + 1 : oidx + 2], pc[:])

            finals(accf, rowWf, NF // 128, 0)
            finals(accc, rowWc, NC_ // 128, 2)

            nc.vector.memset(outb[0:1, 4:8], 0.0)
            nc.sync.dma_start(out_d.ap(), outb[:])

    nc.compile()
    return nc


def _get_program():
    global _PROGRAM
    if _PROGRAM is None:
        _PROGRAM = _build_program()
    return _PROGRAM


def _prep_core_inputs(fine_b, coarse_b, gt_b):
    f16 = np.float16
    xf = np.ones((9, NF), f16)
    xf[0:3] = fine_b.astype(f16).T
    xc = np.ones((9, NC_), f16)
    xc[0:3] = coarse_b.astype(f16).T
    g16 = gt_b.astype(f16)  # [3, M]
    yaug = np.empty((9, M), f16)
    yaug[0:3] = (-2.0 * g16.astype(np.float32)).astype(f16)
    sq = g16.astype(np.float32) ** 2
    hi = sq.astype(f16)
    yaug[3:6] = hi
    yaug[6:9] = (sq - hi.astype(np.float32)).astype(f16)
    # |x|^2 of the fp16-rounded coords, exact fp32, laid out [p, i] = n=128*i+p
    x2f = (fine_b.astype(f16).astype(np.float32) ** 2).sum(1).reshape(-1, 128).T
    x2c = (coarse_b.astype(f16).astype(np.float32) ** 2).sum(1).reshape(-1, 128).T
    return {
        "xaug_f": xf,
        "xaug_c": xc,
        "yaug": yaug,
        "x2f": np.ascontiguousarray(x2f, np.float32),
        "x2c": np.ascontiguousarray(x2c, np.float32),
        "iden": np.eye(128, dtype=f16),
        "ones128": np.ones((128, 1), np.float32),
    }


def kernel(coarse, fine, gt, alpha):
    global LAST_RESULTS
    from concourse import bass_utils

    coarse = np.asarray(coarse, np.float32)
    fine = np.asarray(fine, np.float32)
    gt = np.asarray(gt, np.float32)
    alpha = np.float32(np.asarray(alpha))

    nc = _get_program()
    in_maps = [_prep_core_inputs(fine[b], coarse[b], gt[b]) for b in range(B)]
    res = bass_utils.run_bass_kernel_spmd(
        nc, in_maps, core_ids=list(range(B)), trace=PROFILE
    )
    LAST_RESULTS = res
    per = np.stack([r["out"][0] for r in res.results]).astype(np.float64)  # [B, 8]
    lf = np.float32((per[:, 0] / NF + per[:, 1] / M).mean())
    lc = np.float32((per[:, 2] / NC_ + per[:, 3] / M).mean())
    loss = np.float32(lc + np.float32(alpha) * lf)
    return (loss, lc, lf)


if __name__ == "__main__":
    rng = np.random.default_rng(0)
    out = kernel(
        coarse=rng.standard_normal((B, NC_, 3)).astype(np.float32),
        fine=rng.standard_normal((B, NF, 3)).astype(np.float32),
        gt=rng.standard_normal((B, 3, M)).astype(np.float32),
        alpha=np.float32(1.0),
    )
    print(out)

